# revision 3
# baseline (speedup 1.0000x reference)
"""AdaptiveESN Trainium2 kernel.

Echo State Network: B=64, T=2048, D=128, H=512, leaky a=0.26.
    h_t = (1-a) h_{t-1} + a tanh(x_t W_in^T + b_in + h_{t-1} W_res^T + b_res)
    y_t = h_t W_ro^T

Strategy: TIME-parallel across 8 NeuronCores. The per-step cost on the PE is
weight-load bound (20 LDWEIGHTS of 128x128 per step) and nearly independent
of the rhs width, so each core carries the FULL batch (64 columns) and a
1/8 slice of the sequence. The ESN is contracting (||(1-a)I + a D W_res||_2
<= 0.74 + 0.26*0.18 ~ 0.79), so cores 1..7 recover the true state of their
segment start by running BURN=32 extra steps from h=0 (state error ~5e-6).
Core 0 starts at t=0 exactly. Per core: S = 288 steps instead of 2048.

Per step the scan is the baseline batched matvec z = (a*W_res) h~ + W_in x_t
with h~ = h/a; blend h~_new = (1-a) h~ + tanh(z + b). W_res is stored fp8e4m3
scaled by 2^8 (weight-load at ~2x bf16 FWL rate); the 2^-8 is folded into the
tanh activation's input scale, W_in/bias are pre-scaled by 2^8 to match.
Readout y = (a W_ro) h~ is fused per time-chunk.

Layouts (host-prepped, per core c with segment start t0 = max(0, 256c-32)):
    xt   bf16 [128, S*64]   xt[d, s*64+b] = x[b, t0+s, d]
    wres fp8  [128, 2048]   tile (j,i) at cols (j*4+i)*128: (2^8 a W_res).T
    win  bf16 [128, 512]    (2^8 W_in).T
    wro  bf16 [128, 512]    tile j at cols j*128: (a W_ro).T block
    bias f32  [128, 4]      (b_in + b_res) chunk i in col i   (unscaled)
    out  f32  [128, S*64]   out[d, s*64+b] = y[b, t0+s, d]
Host gather: core 0 keeps steps [0,256), cores >=1 keep [32,288).
"""
import sys

if "/opt/trn_rl_repo" not in sys.path:
    sys.path.insert(0, "/opt/trn_rl_repo")

import numpy as np
import ml_dtypes

import concourse.bass as bass
from concourse import bacc
import concourse.mybir as mybir
import concourse.tile as tile
from concourse.bass_utils import run_bass_kernel_spmd

try:
    import jax

    jax.config.update("jax_compilation_cache_dir", "/tmp/jax_neff_cache")
    jax.config.update("jax_persistent_cache_min_compile_time_secs", 10)
except Exception:
    pass

B, T, D, H = 64, 2048, 128, 512
LEAKY = 0.26
NCORES = 8
SEG = T // NCORES         # output steps per core
BURN = 32                 # burn-in steps for cores 1..7
S = SEG + BURN            # scan steps per core
BL = B                    # batch columns per core (full batch)
NCH = H // 128            # H chunks (partition tiles)
TC = 48                   # time steps per states chunk (S/TC must be even for reps)
W = NCH * BL              # step-major state width
BF16 = mybir.dt.bfloat16
F32 = mybir.dt.float32

TRACE = False             # test harness can flip this for profiling
WRES_FP8 = True           # fp8e4m3 W_res with 2^8 prescale (see module docstring)
FP8_SCALE = 256.0
_last_results = None      # BassKernelResults of the most recent run


def build(s_total=S, tc=TC, reps=1, wres_fp8=WRES_FP8):
    """Build the per-core Bacc graph (same graph on all 8 cores).

    reps > 1 wraps the whole scan in a hardware For_i loop that re-runs it
    (same instruction count) — used to measure pure on-device time via
    wall-clock deltas between two reps values.
    """
    nchunks = s_total // tc
    assert nchunks * tc == s_total
    assert nchunks % 2 == 0 or nchunks == 1 or reps == 1

    nc = bacc.Bacc(None, target_bir_lowering=False)
    xt_e = nc.declare_dram_parameter("xt", [128, s_total * BL], BF16, isOutput=False)
    wres_dt = mybir.dt.float8e4 if wres_fp8 else BF16
    wres_e = nc.declare_dram_parameter("wres", [128, 16 * 128], wres_dt, isOutput=False)
    win_e = nc.declare_dram_parameter("win", [128, NCH * 128], BF16, isOutput=False)
    wro_e = nc.declare_dram_parameter("wro", [128, NCH * 128], BF16, isOutput=False)
    bias_e = nc.declare_dram_parameter("bias", [128, NCH], F32, isOutput=False)
    out_e = nc.declare_dram_parameter("out", [128, s_total * BL], F32, isOutput=True)
    act_scale = (1.0 / FP8_SCALE) if wres_fp8 else 1.0

    with tile.TileContext(nc) as tc_ctx:
        with (
            tc_ctx.tile_pool(name="const", bufs=1) as const_pool,
            tc_ctx.tile_pool(name="p", bufs=8) as p_pool,
            tc_ctx.tile_pool(name="y", bufs=4) as y_pool,
            tc_ctx.tile_pool(name="ostage", bufs=3) as o_pool,
            tc_ctx.tile_pool(name="scan_ps", bufs=6, space=bass.MemorySpace.PSUM) as ps_pool,
            tc_ctx.tile_pool(name="ro_ps", bufs=2, space=bass.MemorySpace.PSUM) as ro_pool,
        ):
            xt_sb = const_pool.tile([128, s_total * BL], BF16)
            wres_sb = const_pool.tile([128, 16 * 128], wres_dt)
            win_sb = const_pool.tile([128, NCH * 128], BF16)
            wro_sb = const_pool.tile([128, NCH * 128], BF16)
            bias_sb = const_pool.tile([128, NCH], F32)
            h0_sb = const_pool.tile([128, W], BF16)
            # states, step-major: column s*W + i*BL + b  (i = H chunk, b = batch)
            st = [
                const_pool.tile([128, tc * W], BF16, name=f"st{k}", tag=f"st{k}")
                for k in range(2)
            ]

            nc.sync.dma_start(wres_sb[:], wres_e[:])
            nc.sync.dma_start(win_sb[:], win_e[:])
            nc.sync.dma_start(wro_sb[:], wro_e[:])
            nc.sync.dma_start(bias_sb[:], bias_e[:])
            nc.sync.dma_start(xt_sb[:], xt_e[:])
            nc.vector.memset(h0_sb[:], 0.0)

            def scan_body(_iv=None):
                for c in range(nchunks):
                    cur, prv = c % 2, (c - 1) % 2
                    for s in range(tc):
                        t = c * tc + s
                        if t == 0:
                            hprev = h0_sb[:]
                        elif s == 0:
                            hprev = st[prv][:, (tc - 1) * W : tc * W]
                        else:
                            hprev = st[cur][:, (s - 1) * W : s * W]

                        def hcol(j):
                            return hprev[:, j * BL : (j + 1) * BL]

                        xcol = xt_sb[:, t * BL : (t + 1) * BL]
                        # y = (1-a) * h_{t-1}, all chunks in one DVE op (off-path)
                        y_t = y_pool.tile([128, W], BF16)
                        nc.vector.tensor_scalar_mul(y_t[:], hprev, 1.0 - LEAKY)
                        for i in range(NCH):
                            ps = ps_pool.tile([128, BL], F32)
                            # j-order (0,1,2,win,3): defer the h[3] consumption
                            ops = [
                                (wres_sb[:, (0 * NCH + i) * 128 : (0 * NCH + i + 1) * 128], hcol(0)),
                                (wres_sb[:, (1 * NCH + i) * 128 : (1 * NCH + i + 1) * 128], hcol(1)),
                                (wres_sb[:, (2 * NCH + i) * 128 : (2 * NCH + i + 1) * 128], hcol(2)),
                                (win_sb[:, i * 128 : (i + 1) * 128], xcol),
                                (wres_sb[:, (3 * NCH + i) * 128 : (3 * NCH + i + 1) * 128], hcol(3)),
                            ]
                            for k, (lhsT, rhs) in enumerate(ops):
                                nc.tensor.matmul(
                                    ps[:], lhsT, rhs,
                                    start=(k == 0), stop=(k == len(ops) - 1))
                            st_col = st[cur][:, s * W + i * BL : s * W + (i + 1) * BL]
                            p_t = p_pool.tile([128, BL], BF16)
                            nc.scalar.activation(
                                p_t[:], ps[:], mybir.ActivationFunctionType.Tanh,
                                bias=bias_sb[:, i : i + 1], scale=act_scale,
                            )
                            nc.vector.tensor_tensor(
                                st_col,
                                y_t[:, i * BL : (i + 1) * BL], p_t[:],
                                op=mybir.AluOpType.add,
                            )
                    # fused readout of chunk c: out = (a W_ro).T @ states
                    base = c * tc * BL
                    st_v = st[cur].rearrange("p (s w) -> p s w", w=W)
                    ns = 512 // BL  # steps per readout tile
                    for n in range(0, tc, ns):
                        nw = min(ns, tc - n)
                        rps = ro_pool.tile([128, 512], F32)
                        for j in range(NCH):
                            nc.tensor.matmul(
                                rps[:, : nw * BL],
                                wro_sb[:, j * 128 : (j + 1) * 128],
                                st_v[:, n : n + nw, j * BL : (j + 1) * BL],
                                start=(j == 0),
                                stop=(j == NCH - 1),
                            )
                        ostage = o_pool.tile([128, 512], F32)
                        nc.scalar.activation(
                            ostage[:, : nw * BL], rps[:, : nw * BL],
                            mybir.ActivationFunctionType.Copy,
                        )
                        nc.sync.dma_start(
                            out_e[:, base + n * BL : base + (n + nw) * BL],
                            ostage[:, : nw * BL],
                        )

            if reps == 1:
                scan_body()
            else:
                with tc_ctx.For_i(0, reps, 1) as _i:
                    scan_body(_i)

    nc.compile()
    return nc


def host_prep(x, W_in, b_in, W_res, b_res, W_ro, wres_fp8=WRES_FP8):
    """Produce the per-core in_maps (host-side layout/dtype prep only)."""
    a = np.float32(LEAKY)
    wscale = np.float32(FP8_SCALE) if wres_fp8 else np.float32(1.0)
    AT = (wscale * a * W_res).T.astype(np.float32)            # [in, out]
    wres_np_dt = ml_dtypes.float8_e4m3 if wres_fp8 else ml_dtypes.bfloat16
    wres = (
        AT.reshape(NCH, 128, NCH, 128).transpose(1, 0, 2, 3).reshape(128, 16 * 128)
    ).astype(wres_np_dt)
    win = (wscale * W_in).T.astype(ml_dtypes.bfloat16)        # [128, 512]
    R = (a * W_ro).T.astype(np.float32)                       # [512, 128]
    wro = R.reshape(NCH, 128, 128).transpose(1, 0, 2).reshape(128, NCH * 128).astype(
        ml_dtypes.bfloat16
    )
    bias = (b_in + b_res).astype(np.float32).reshape(NCH, 128).T.copy()  # [128, 4]

    in_maps = []
    for c in range(NCORES):
        t0 = 0 if c == 0 else c * SEG - BURN
        xl = x[:, t0 : t0 + S, :]                             # [64, S, 128]
        xt = np.ascontiguousarray(xl.transpose(2, 1, 0).reshape(128, S * BL))
        in_maps.append({
            "xt": xt.astype(ml_dtypes.bfloat16),
            "wres": wres, "win": win, "wro": wro, "bias": bias,
        })
    return in_maps


_nc_cache = {}


def kernel(x, W_in, b_in, W_res, b_res, W_ro):
    """Full inputs in, full output out ([B, T, D] float32)."""
    global _last_results
    x, W_in, b_in, W_res, b_res, W_ro = (
        np.asarray(t, dtype=np.float32) for t in (x, W_in, b_in, W_res, b_res, W_ro)
    )
    assert x.shape == (B, T, D)
    if "nc" not in _nc_cache:
        _nc_cache["nc"] = build()
    nc = _nc_cache["nc"]

    in_maps = host_prep(x, W_in, b_in, W_res, b_res, W_ro)
    res = run_bass_kernel_spmd(nc, in_maps, list(range(NCORES)), trace=TRACE)
    _last_results = res

    out = np.empty((B, T, D), dtype=np.float32)
    for c in range(NCORES):
        oc = res.results[c]["out"].reshape(128, S, BL)        # [d, s, b]
        s0 = 0 if c == 0 else BURN
        out[:, c * SEG : (c + 1) * SEG] = oc[:, s0 : s0 + SEG].transpose(2, 1, 0)
    return out
